# revision 14
# baseline (speedup 1.0000x reference)
"""Trainium2 Bass kernel for nn_AdaptiveAngleConv.

Reference computes, for each of 5 angles, a bilinear "deformable" 3x3
sampling of x (2,256,64,64) into a (2,256,192,192) image, then a 3x3
VALID conv (stride 1) with a shared weight (256,256,3,3), giving 5
outputs of (2,256,190,190).

Key math: the reference's clipped bilinear sampling is exactly an
UNclipped separable 2x2 stencil with constant per-(angle, n)
coefficients on a zero-padded x — every clipped index lands on a
zero-pad row/col, so the clip never changes a nonzero contribution.
Angles 0/90/180 have integer offsets; 45/135 need a 2-pass (rows then
cols) lerp with the shifted-difference trick so each pass is one fused
scalar_tensor_tensor op per distinct offset.

Sharding: output rows are split across the 8 cores (24 rows each, 8*24
= 192 >= 190). Each core receives a pre-sliced 13-row input slab so the
SPMD graph is identical on every core; no collectives.

Conv structure per job (one (angle, batch)):
- angle 0: phase-collapsed taps (g(m) = m//3 + m%3 - 1 collides for
  m=1,3 / m=2,4) -> 49 taps per phase grid vs 81, using host-summed
  weights, reading the input slab directly.
- angles 90/180: integer offsets mean every output phase (rho,sig) is
  a pure 9-tap conv on the slab with remapped taps
  (R,C) = (alpha_i + s_j - 1, beta_j + 1 - r_i) for 90 and the
  separable (alpha_i + 1 - r_i, beta_j + 1 - s_j) for 180 — so these
  jobs read the slab directly too (no sampled-image build at all).
- angles 45/135: 81 taps on the vector-built lerped image xo.

All matmuls are fp16 (same 1 cyc/row PE rate as bf16, ~8x better
rounding); accumulation groups are ct-outer so the first matmuls of a
job depend only on the first half of the weights/slab DMAs. Outputs
are staged and DMA'd as fp16 (host casts back to f32), halving output
HBM traffic and the end-of-kernel DMA tail. The schedule opens with
the slab-direct angle-90 job (smallest DMA dependency set) behind a
chain of small warm-up matmuls that hold the PE clock gate at 8/8
during the input-DMA window, and closes with the staggered per-rho
DMAs of the collapsed angle-0 job.
"""

import os
import sys

for _p in ("/opt/trn_rl_repo", "/root/.axon_site/_ro/trn_rl_repo"):
    if os.path.isdir(_p) and _p not in sys.path:
        sys.path.insert(0, _p)

import numpy as np

import concourse.bass as bass
import concourse.mybir as mybir
from concourse import bacc, tile
from concourse.alu_op_type import AluOpType
from concourse.bass_utils import run_bass_kernel_spmd

F32 = mybir.dt.float32
BF16 = mybir.dt.float16  # fp16: same 1 cyc/row PE rate as bf16, 3 more mantissa bits

S2 = 2 ** 0.5
ANGLES = [0, 45, 90, 135, 180]
_OFF = {
    0: ([0.0] * 9, [0.0] * 9),
    1: ([1 - S2, 1 - S2 * 0.5, 1, -S2 * 0.5, 0, S2 * 0.5, -1, S2 * 0.5 - 1, S2 - 1],
        [1, S2 * 0.5, S2 - 1, 1 - S2 * 0.5, 0, S2 * 0.5 - 1, 1 - S2, -S2 * 0.5, -1]),
    2: ([0, 1, 2, -1, 0, 1, -2, -1, 0],
        [2, 1, 0, 1, 0, -1, 0, -1, -2]),
    3: ([1, 1 + S2 * 0.5, 1 + S2, -S2 * 0.5, 0, S2 * 0.5, -1 - S2, -1 - S2 * 0.5, -1],
        [1 + S2, S2 * 0.5, -1, 1 + S2 * 0.5, 0, -1 - S2 * 0.5, 1, -S2 * 0.5, 1 + S2]),
    4: ([2, 2, 2, 0, 0, 0, -2, -2, -2],
        [2, 0, -2, 2, 0, -2, 2, 0, -2]),
}

NCORES = 8
NR = 24            # output rows per core (8*24 = 192, rows 190/191 dropped)
SLAB_ROWS = 13     # input rows a core needs: hi in [8k-2, 8k+10]
SLAB_COLS = 70     # data cols -2..67
SLABF = SLAB_ROWS * SLAB_COLS  # 910
XO_ROWS = 26       # NR + 2 halo rows of the sampled image
XO_F = XO_ROWS * 192


def _tables():
    """Per angle: list of (n, r, s, Ax, fx, Ay, fy) in f32 semantics."""
    rng = np.arange(-1, 2)
    pnx, pny = np.meshgrid(rng, rng, indexing="ij")
    pnx = pnx.reshape(-1).astype(np.float32)
    pny = pny.reshape(-1).astype(np.float32)
    out = {}
    for a in ANGLES:
        ox, oy = _OFF[a // 45]
        dx = pnx + np.array(ox, dtype=np.float32)
        dy = pny + np.array(oy, dtype=np.float32)
        rows = []
        for n in range(9):
            Ax = int(np.floor(dx[n]))
            Ay = int(np.floor(dy[n]))
            fx = float(np.float32(dx[n] - Ax))
            fy = float(np.float32(dy[n] - Ay))
            rows.append((n, n // 3, n % 3, Ax, fx, Ay, fy))
        out[a] = rows
    return out


TABLES = _tables()
# distinct fractional row offsets shared by the 45/135 pair
LERP_DS = sorted({(t[3], t[4]) for a in (45, 135) for t in TABLES[a]})

# Integer-offset angles 90/180: per output phase (rho,sig), conv tap
# (ki,kj) reads slab offset (R,C). dx/dy for n=3r+s: 90 -> (s-1, 1-r),
# 180 -> (1-r, 1-s).
def _phase_taps(a):
    taps = {}
    for rho in range(3):
        for sig in range(3):
            lst = []
            for ki in range(3):
                ai, ri = divmod(rho + ki, 3)
                for kj in range(3):
                    bj, sj = divmod(sig + kj, 3)
                    if ri == 1 and sj == 1:
                        continue  # shared center tap
                    if a == 90:
                        R, C = ai + (sj - 1), bj + (1 - ri)
                    else:  # 180
                        R, C = ai + (1 - ri), bj + (1 - sj)
                    lst.append((3 * ki + kj, R, C))
            taps[(rho, sig)] = lst
    return taps


PHASE_TAPS = {90: _phase_taps(90), 180: _phase_taps(180)}

# The center sampling tap (ri=1, sj=1 -> n=4) reads x itself with the same
# weight kk and the same (ai,bj) window for EVERY angle: compute it once per
# (b, ot, phase) and fuse the add into each angle's PSUM evacuation.
def _center(rho, sig):
    ki, kj = (1 - rho) % 3, (1 - sig) % 3
    return 3 * ki + kj, (rho + ki) // 3, (sig + kj) // 3


CENTER = {(rho, sig): _center(rho, sig) for rho in range(3) for sig in range(3)}

# Angle-0 phase-collapsed conv: output phase rho uses row taps di with the
# listed combo of original kernel rows (g(m)=m//3+m%3-1 collides for m=1,3
# and m=2,4). Combo indices into the host-precomputed sums: 0,1,2 = single
# ki, 3 = ki0+ki2. Same structure for columns. 49 taps/phase-grid vs 81.
ROW_COMBOS = [(0,), (1,), (2,), (0, 2)]
PHROWS = {0: [(-1, 0), (0, 1), (1, 2)],
          1: [(0, 3), (1, 1)],
          2: [(1, 3), (0, 1)]}

N_WARM = 14        # warm-up matmuls: keep HAM at 8/8 through the DMA window
# valid output columns per column phase sig (cols sig::3 below 190)
NCOL = {0: 64, 1: 63, 2: 63}
SIG_OFF = {0: 0, 1: 8 * 64, 2: 8 * 64 + 8 * 63}  # sig block offsets in a rho block
ROWB = 8 * 190     # elements per rho block in packed staging (=sum of sig blocks)


def build_graph():
    nc = bacc.Bacc()
    # xc: per ct, the opening half-job's whole dependency set in ONE
    # DMA: [slab b0 | all 9 ot=0 weight slices]. The early window is
    # descriptor-rate limited; one descriptor per partition wins.
    xc = nc.declare_dram_parameter("xc", [2, 128, SLABF + 1152], BF16, False)
    xs1 = nc.declare_dram_parameter("xs1", [2, 128, SLABF], BF16, False)
    wt = nc.declare_dram_parameter("wt", [2, 128, 9 * 128], BF16, False)  # ot=1
    wc = nc.declare_dram_parameter("wc", [2, 128, 16 * 2 * 128], BF16, False)
    # flat per-(angle,b,ot) rows: packed rho-major [3,8,190] for the
    # slab-direct jobs (contiguous DMA), natural [24,190] for 45/135.
    out = nc.declare_dram_parameter("out", [5, 2, 2, 128, 3 * 8 * 190], BF16, True)

    with tile.TileContext(nc) as tc:
        with (
            tc.tile_pool(name="const", bufs=1) as constp,
            tc.tile_pool(name="cenp", bufs=1) as cenp,
            tc.tile_pool(name="xop", bufs=2) as xop,
            tc.tile_pool(name="rcp", bufs=1) as rcp,
            tc.tile_pool(name="stg", bufs=2) as stgp,
            tc.tile_pool(name="ps", bufs=8, space="PSUM") as psp,
        ):
            # HAM warm-up: dependency-free matmuls on a vector-memset tile
            # keep the PE busy during the input-DMA window so the clock
            # gate is already at 8/8 when the first real matmul issues.
            # N=256 keeps the handoff granularity small. Results land in
            # a scratch PSUM bank and are never read.
            warm = constp.tile([128, 384], BF16, name="warm", tag="warm")
            nc.gpsimd.memset(warm[:], 0.0)
            wps = psp.tile([128, 512], F32, name="wps", tag="ps")
            for _ in range(N_WARM):
                nc.tensor.matmul(wps[:, :256], warm[:, :128], warm[:, 128:384],
                                 start=True, stop=True)

            # DMA order matters for the head: the first job (slab-direct
            # angle-90, batch 0, ct-outer) can issue its first matmul once
            # slab b0/ct0 and the first wt chunk (ct0, ot0, kk 0-2) have
            # landed.
            slab = {}

            # Inputs split across BOTH hardware-DGE queues: sync carries
            # ct=0, the scalar engine's queue carries ct=1. The packed
            # critical xc goes first on each queue.
            eng = {0: nc.sync, 1: nc.scalar}
            xc_sb = []
            w_sb = []
            wc_sb = []
            for ct in range(2):
                t = constp.tile([128, SLABF + 1152], BF16,
                                name=f"xc{ct}", tag=f"xc{ct}")
                eng[ct].dma_start(t[:], xc[ct])
                xc_sb.append(t)
                slab[(0, ct)] = t[:, :SLABF]
                w_sb.append(constp.tile([128, 9 * 128], BF16,
                                        name=f"w{ct}", tag=f"w{ct}"))
            for ct in range(2):  # collapsed weights, ot=0 half first
                wctile = constp.tile([128, 16 * 2 * 128], BF16, name=f"wc{ct}",
                                     tag=f"wc{ct}")
                eng[ct].dma_start(wctile[:, :2048], wc[ct][:, :2048])
                wc_sb.append(wctile)
            for ct in range(2):  # ot=1 weights
                eng[ct].dma_start(w_sb[ct][:], wt[ct])
            for ct in range(2):  # batch-1 slabs
                s = constp.tile([128, SLABF], BF16,
                                name=f"slab1{ct}", tag=f"slab1{ct}")
                eng[ct].dma_start(s[:], xs1[ct])
                slab[(1, ct)] = s
            for ct in range(2):
                eng[ct].dma_start(wc_sb[ct][:, 2048:], wc[ct][:, 2048:])

            def slab3(b, ct):
                return slab[(b, ct)].rearrange("p (r c) -> p r c", c=SLAB_COLS)

            def wap(ct, ot, kk):
                if ot == 0:
                    return xc_sb[ct][:, SLABF + kk * 128 : SLABF + (kk + 1) * 128]
                return w_sb[ct][:, kk * 128 : (kk + 1) * 128]

            def build_lerp_rc(b):
                """Shared row-lerp R_d and col-diff C_d tiles for 45+135."""
                R = {}
                C = {}
                for ct in range(2):
                    sv = slab3(b, ct)
                    dr = rcp.tile([128, 12 * SLAB_COLS], BF16,
                                  name=f"dr{ct}", tag=f"dr{ct}")
                    drv = dr.rearrange("p (r c) -> p r c", c=SLAB_COLS)
                    nc.vector.tensor_tensor(drv, sv[:, 1:13, :], sv[:, 0:12, :],
                                            AluOpType.subtract)
                    for di, (Ax, fx) in enumerate(LERP_DS):
                        if fx == 0.0:
                            rv = sv[:, 2 + Ax : 11 + Ax, :]
                        else:
                            rt = rcp.tile([128, 9 * SLAB_COLS], BF16,
                                          name=f"r{ct}_{di}", tag=f"r{ct}_{di}")
                            rv = rt.rearrange("p (r c) -> p r c", c=SLAB_COLS)
                            nc.vector.scalar_tensor_tensor(
                                rv, drv[:, 2 + Ax : 11 + Ax, :], fx,
                                sv[:, 2 + Ax : 11 + Ax, :],
                                AluOpType.mult, AluOpType.add)
                        ctile = rcp.tile([128, 9 * SLAB_COLS], BF16,
                                         name=f"c{ct}_{di}", tag=f"c{ct}_{di}")
                        cv = ctile.rearrange("p (r c) -> p r c", c=SLAB_COLS)
                        nc.vector.tensor_tensor(cv[:, :, 0:69], rv[:, :, 1:70],
                                                rv[:, :, 0:69], AluOpType.subtract)
                        R[(ct, Ax, fx)] = rv
                        C[(ct, Ax, fx)] = cv
                return R, C

            def build_xo_lerp(a, b, R, C):
                """Phase-major sampled image: 9 contiguous blocks of
                [9 rows x 65 cols], one per sampling phase n, so the conv
                reads contiguous-inner APs and the builds write contiguous
                (fast DVE mode). Block r=2 only needs 8 rows."""
                xo = []
                for ct in range(2):
                    t = xop.tile([128, 9 * 585], BF16, name=f"xo{ct}", tag=f"xo{ct}")
                    for (n, r, s, Ax, fx, Ay, fy) in TABLES[a]:
                        if n == 4:
                            continue  # center block replaced by shared partial
                        nrow = 9 if r < 2 else 8
                        dst = t[:, n * 585 : n * 585 + nrow * 65].rearrange(
                            "p (r c) -> p r c", c=65)
                        rv = R[(ct, Ax, fx)][:, :nrow, 2 + Ay : 67 + Ay]
                        if fy == 0.0:
                            nc.vector.tensor_copy(dst, rv)
                        else:
                            cv = C[(ct, Ax, fx)][:, :nrow, 2 + Ay : 67 + Ay]
                            nc.vector.scalar_tensor_tensor(
                                dst, cv, fy, rv,
                                AluOpType.mult, AluOpType.add)
                    xo.append(t)
                return xo

            cen = {}

            def emit_center(b, ot):
                """Shared center-tap partials for one (b, ot): 9 phase
                windows, 2 ct matmuls each, Act-evacuated to SBUF f16."""
                t = cenp.tile([128, 9 * 512], BF16, name=f"cen{b}{ot}",
                              tag=f"cen{b}{ot}")
                for rho in range(3):
                    for sig in range(3):
                        kk, ai, bj = CENTER[(rho, sig)]
                        nn = NCOL[sig]
                        ps = psp.tile([128, 512], F32, name="cps", tag="ps")
                        for ct in range(2):
                            sv = slab3(b, ct)
                            nc.tensor.matmul(
                                ps[:, : 8 * nn], wap(ct, ot, kk),
                                sv[:, ai + 2 : ai + 10, bj + 2 : bj + 2 + nn],
                                start=(ct == 0), stop=(ct == 1))
                        o = (3 * rho + sig) * 512
                        nc.scalar.copy(t[:, o : o + 8 * nn], ps[:, : 8 * nn])
                cen[(b, ot)] = t

            def _evac_rho_cen(stg, ps, rho, dma_dst, cent):
                """Evacuate one rho group, fusing the shared center-tap add
                on the vector engine."""
                for sig in range(3):
                    nn = NCOL[sig]
                    o = rho * ROWB + SIG_OFF[sig]
                    co = (3 * rho + sig) * 512
                    nc.vector.tensor_tensor(
                        stg[:, o : o + 8 * nn], ps[sig][:, : 8 * nn],
                        cent[:, co : co + 8 * nn], AluOpType.add)
                nc.sync.dma_start(
                    dma_dst[:, rho * ROWB : (rho + 1) * ROWB],
                    stg[:, rho * ROWB : (rho + 1) * ROWB])

            def _evac_rho(stg, ps, rho, last, dma_dst):
                """Evacuate one rho group's 3 column phases into packed
                phase-major staging (contiguous copies — the host
                de-interleaves columns) and DMA the rho block. The closing
                job evacuates on the otherwise-idle vector engine and DMAs
                per sig block, so only the last sig's short chain trails
                the final matmul."""
                for sig in range(3):
                    nn = NCOL[sig]
                    o = rho * ROWB + SIG_OFF[sig]
                    dst = stg[:, o : o + 8 * nn]
                    if last:
                        nc.vector.tensor_copy(dst, ps[sig][:, : 8 * nn])
                        nc.sync.dma_start(dma_dst[:, o : o + 8 * nn], dst)
                    else:
                        nc.scalar.copy(dst, ps[sig][:, : 8 * nn])
                if not last:
                    nc.sync.dma_start(
                        dma_dst[:, rho * ROWB : (rho + 1) * ROWB],
                        stg[:, rho * ROWB : (rho + 1) * ROWB])

            def conv_job_angle0(ai, b, last=False):
                """Phase-collapsed conv reading the slab directly (no xo).
                Groups by output row phase rho so each rho's rows can DMA
                out as soon as its three column phases are evacuated."""
                for ot in range(2):
                    stg = stgp.tile([128, 3 * ROWB], BF16, name="stg", tag="stg")
                    for rho in range(3):
                        ps = {sig: psp.tile([128, 512], F32, name=f"ps{sig}",
                                            tag="ps") for sig in range(3)}
                        nmm = {sig: len(PHROWS[rho]) * len(PHROWS[sig]) * 2
                               for sig in range(3)}
                        cnt = {0: 0, 1: 0, 2: 0}
                        for ct in range(2):
                            sv = slab3(b, ct)
                            for (di, ri) in PHROWS[rho]:
                                for ci in range(4):
                                    cb = ri * 4 + ci
                                    w_ap = wc_sb[ct][:, (ot * 16 + cb) * 128 :
                                                     (ot * 16 + cb + 1) * 128]
                                    for sig in range(3):
                                        djs = [dj for (dj, c) in PHROWS[sig]
                                               if c == ci]
                                        if not djs:
                                            continue
                                        dj = djs[0]
                                        nn = NCOL[sig]
                                        i = cnt[sig]
                                        nc.tensor.matmul(
                                            ps[sig][:, : 8 * nn], w_ap,
                                            sv[:, di + 2 : di + 10,
                                               dj + 2 : dj + 2 + nn],
                                            start=(i == 0),
                                            stop=(i == nmm[sig] - 1))
                                        cnt[sig] = i + 1
                        _evac_rho(stg, ps, rho, last, out[ai, b, ot])

            def conv_job_phase(ai, b, angle, emit_cen=False):
                """Slab-direct conv for integer-offset angles 90/180: each
                output phase is an 8-tap conv with remapped slab offsets
                plus the shared center partial added at evacuation."""
                ptaps = PHASE_TAPS[angle]
                for ot in range(2):
                    if emit_cen:
                        emit_center(b, ot)
                    stg = stgp.tile([128, 3 * ROWB], BF16, name="stg", tag="stg")
                    for rho in range(3):
                        ps = {sig: psp.tile([128, 512], F32, name=f"ps{sig}",
                                            tag="ps") for sig in range(3)}
                        # kk-outer / sig-inner: matmuls sharing a stationary
                        # weight run back-to-back so the PE skips reloads.
                        tapm = {sig: {kk: (R, C)
                                      for (kk, R, C) in ptaps[(rho, sig)]}
                                for sig in range(3)}
                        cnt = {0: 0, 1: 0, 2: 0}
                        for ct in range(2):
                            sv = slab3(b, ct)
                            for kk in range(9):
                                w_ap = wap(ct, ot, kk)
                                for sig in range(3):
                                    if kk not in tapm[sig]:
                                        continue
                                    R, C = tapm[sig][kk]
                                    nn = NCOL[sig]
                                    i = cnt[sig]
                                    nc.tensor.matmul(
                                        ps[sig][:, : 8 * nn], w_ap,
                                        sv[:, R + 2 : R + 10, C + 2 : C + 2 + nn],
                                        start=(i == 0), stop=(i == 15))
                                    cnt[sig] = i + 1
                        _evac_rho_cen(stg, ps, rho, out[ai, b, ot],
                                      cen[(b, ot)])

            def conv_job(ai, b, xo):
                """Phase-structured conv on the phase-major sampled image;
                same shape as conv_job_phase but taps read xo blocks."""
                xov = [xo[ct].rearrange("p (n r c) -> p n r c", n=9, c=65)
                       for ct in range(2)]
                for ot in range(2):
                    stg = stgp.tile([128, 3 * ROWB], BF16, name="stg", tag="stg")
                    for rho in range(3):
                        ps = {sig: psp.tile([128, 512], F32, name=f"ps{sig}",
                                            tag="ps") for sig in range(3)}
                        cnt = {0: 0, 1: 0, 2: 0}
                        for ct in range(2):
                            for ki in range(3):
                                ai_, ri = divmod(rho + ki, 3)
                                for kj in range(3):
                                    kk = 3 * ki + kj
                                    w_ap = wap(ct, ot, kk)
                                    for sig in range(3):
                                        bj, sj = divmod(sig + kj, 3)
                                        if ri == 1 and sj == 1:
                                            continue  # shared center tap
                                        n = 3 * ri + sj
                                        nn = NCOL[sig]
                                        i = cnt[sig]
                                        nc.tensor.matmul(
                                            ps[sig][:, : 8 * nn], w_ap,
                                            xov[ct][:, n, ai_ : ai_ + 8,
                                                    bj : bj + nn],
                                            start=(i == 0), stop=(i == 15))
                                        cnt[sig] = i + 1
                        _evac_rho_cen(stg, ps, rho, out[ai, b, ot],
                                      cen[(b, ot)])

            # Slab-direct angle 90 first (smallest DMA dependency set),
            # collapsed angle 0 last for b=1 so the tail is its staggered
            # per-rho DMAs.
            conv_job_phase(2, 0, 90, emit_cen=True)
            conv_job_angle0(0, 0)
            conv_job_phase(4, 0, 180)
            R, C = build_lerp_rc(0)
            xo = build_xo_lerp(45, 0, R, C)
            conv_job(1, 0, xo)
            xo = build_xo_lerp(135, 0, R, C)
            conv_job(3, 0, xo)
            conv_job_phase(2, 1, 90, emit_cen=True)
            conv_job_phase(4, 1, 180)
            R, C = build_lerp_rc(1)
            xo = build_xo_lerp(45, 1, R, C)
            conv_job(1, 1, xo)
            xo = build_xo_lerp(135, 1, R, C)
            conv_job(3, 1, xo)
            conv_job_angle0(0, 1, last=True)

    nc.compile()
    return nc


_GRAPH = None


def _graph():
    global _GRAPH
    if _GRAPH is None:
        _GRAPH = build_graph()
    return _GRAPH


def prep_inputs(x, weight):
    x = np.asarray(x, dtype=np.float32)
    weight = np.asarray(weight, dtype=np.float32)
    # pad data rows -2..66, cols -2..67
    xp = np.pad(x, ((0, 0), (0, 0), (2, 3), (2, 4))).astype(np.float16)
    w6 = weight.reshape(2, 128, 2, 128, 3, 3)                 # [ot,o,ct,c,ki,kj]
    # wt ot-major per ct: [ct, c, ot, ki, kj, o]
    w = w6.transpose(2, 3, 0, 4, 5, 1).reshape(2, 128, 2 * 9 * 128)
    w = np.ascontiguousarray(w.astype(np.float16))
    w0, w1 = w[:, :, :1152], np.ascontiguousarray(w[:, :, 1152:])
    xc_cores, xs1_cores = [], []
    for k in range(NCORES):
        sl = xp[:, :, 8 * k : 8 * k + SLAB_ROWS, :]          # [2,256,13,70]
        sl = sl.reshape(2, 2, 128, SLABF)
        xc_cores.append(np.ascontiguousarray(
            np.concatenate([sl[0], w0], axis=-1)))
        xs1_cores.append(np.ascontiguousarray(sl[1]))
    combos = []
    for Rc in ROW_COMBOS:
        for Cc in ROW_COMBOS:
            combos.append(w6[..., list(Rc), :][..., list(Cc)].sum(axis=(-1, -2)))
    wcarr = np.stack(combos, axis=0)                          # [16,ot,o,ct,c]
    wcarr = wcarr.transpose(3, 4, 1, 0, 2).reshape(2, 128, 2 * 16 * 128)
    wcarr = np.ascontiguousarray(wcarr.astype(np.float16))
    return xc_cores, xs1_cores, w1, wcarr


def assemble(results):
    full = np.empty((5, 2, 256, NCORES * NR, 190), np.float32)
    for k in range(NCORES):
        o = results[k]["out"]                       # [5,2,2,128,3*8*190] f16
        r0 = NR * k
        for ai in range(5):
            a = o[ai].reshape(2, 256, 3 * ROWB)
            # packed phase-major: [rho][sig-block][8 rows][ncol]
            for rho in range(3):
                for sig in range(3):
                    nn = NCOL[sig]
                    off = rho * ROWB + SIG_OFF[sig]
                    blk = a[:, :, off : off + 8 * nn].reshape(2, 256, 8, nn)
                    full[ai, :, :, r0 + rho : r0 + NR : 3, sig::3] = blk
    full = full[:, :, :, :190, :]
    return tuple(np.ascontiguousarray(full[i]) for i in range(5))


def run(x, weight, trace=False, **trace_kw):
    xc_cores, xs1_cores, w1, wcarr = prep_inputs(x, weight)
    nc = _graph()
    in_maps = [{"xc": xc_cores[k], "xs1": xs1_cores[k], "wt": w1, "wc": wcarr}
               for k in range(NCORES)]
    res = run_bass_kernel_spmd(nc, in_maps, core_ids=list(range(NCORES)),
                               trace=trace, **trace_kw)
    return assemble(res.results), res


def kernel(x, weight):
    return run(x, weight)[0]



# revision 17
# speedup vs baseline: 1.1976x; 1.1976x over previous
"""Trainium2 Bass kernel for nn_AdaptiveAngleConv.

Reference computes, for each of 5 angles, a bilinear "deformable" 3x3
sampling of x (2,256,64,64) into a (2,256,192,192) image, then a 3x3
VALID conv (stride 1) with a shared weight (256,256,3,3), giving 5
outputs of (2,256,190,190).

Key math: the reference's clipped bilinear sampling is exactly an
UNclipped separable 2x2 stencil with constant per-(angle, n)
coefficients on a zero-padded x — every clipped index lands on a
zero-pad row/col, so the clip never changes a nonzero contribution.
Angles 0/90/180 have integer offsets; 45/135 need a 2-pass (rows then
cols) lerp with the shifted-difference trick so each pass is one fused
scalar_tensor_tensor op per distinct offset.

Sharding: output rows are split across the 8 cores (24 rows each, 8*24
= 192 >= 190). Each core receives a pre-sliced 13-row input slab so the
SPMD graph is identical on every core; no collectives.

Conv structure per job (one (angle, batch)):
- angle 0: phase-collapsed taps (g(m) = m//3 + m%3 - 1 collides for
  m=1,3 / m=2,4) -> 49 taps per phase grid vs 81, using host-summed
  weights, reading the input slab directly.
- angles 90/180: integer offsets mean every output phase (rho,sig) is
  a pure 9-tap conv on the slab with remapped taps
  (R,C) = (alpha_i + s_j - 1, beta_j + 1 - r_i) for 90 and the
  separable (alpha_i + 1 - r_i, beta_j + 1 - s_j) for 180 — so these
  jobs read the slab directly too (no sampled-image build at all).
- angles 45/135: 81 taps on the vector-built lerped image xo.

All matmuls are fp16 (same 1 cyc/row PE rate as bf16, ~8x better
rounding); accumulation groups are ct-outer so the first matmuls of a
job depend only on the first half of the weights/slab DMAs. Outputs
are staged and DMA'd as fp16 (host casts back to f32), halving output
HBM traffic and the end-of-kernel DMA tail. The schedule opens with
the slab-direct angle-90 job (smallest DMA dependency set) behind a
chain of small warm-up matmuls that hold the PE clock gate at 8/8
during the input-DMA window, and closes with the staggered per-rho
DMAs of the collapsed angle-0 job.
"""

import os
import sys

for _p in ("/opt/trn_rl_repo", "/root/.axon_site/_ro/trn_rl_repo"):
    if os.path.isdir(_p) and _p not in sys.path:
        sys.path.insert(0, _p)

import numpy as np

import concourse.bass as bass
import concourse.mybir as mybir
from concourse import bacc, tile
from concourse.alu_op_type import AluOpType
from concourse.bass_utils import run_bass_kernel_spmd

F32 = mybir.dt.float32
BF16 = mybir.dt.float16  # fp16: same 1 cyc/row PE rate as bf16, 3 more mantissa bits

S2 = 2 ** 0.5
ANGLES = [0, 45, 90, 135, 180]
_OFF = {
    0: ([0.0] * 9, [0.0] * 9),
    1: ([1 - S2, 1 - S2 * 0.5, 1, -S2 * 0.5, 0, S2 * 0.5, -1, S2 * 0.5 - 1, S2 - 1],
        [1, S2 * 0.5, S2 - 1, 1 - S2 * 0.5, 0, S2 * 0.5 - 1, 1 - S2, -S2 * 0.5, -1]),
    2: ([0, 1, 2, -1, 0, 1, -2, -1, 0],
        [2, 1, 0, 1, 0, -1, 0, -1, -2]),
    3: ([1, 1 + S2 * 0.5, 1 + S2, -S2 * 0.5, 0, S2 * 0.5, -1 - S2, -1 - S2 * 0.5, -1],
        [1 + S2, S2 * 0.5, -1, 1 + S2 * 0.5, 0, -1 - S2 * 0.5, 1, -S2 * 0.5, 1 + S2]),
    4: ([2, 2, 2, 0, 0, 0, -2, -2, -2],
        [2, 0, -2, 2, 0, -2, 2, 0, -2]),
}

NCORES = 8
NR = 24            # output rows per core (8*24 = 192, rows 190/191 dropped)
SLAB_ROWS = 13     # input rows a core needs: hi in [8k-2, 8k+10]
SLAB_COLS = 70     # data cols -2..67
SLABF = SLAB_ROWS * SLAB_COLS  # 910
XO_ROWS = 26       # NR + 2 halo rows of the sampled image
XO_F = XO_ROWS * 192


def _tables():
    """Per angle: list of (n, r, s, Ax, fx, Ay, fy) in f32 semantics."""
    rng = np.arange(-1, 2)
    pnx, pny = np.meshgrid(rng, rng, indexing="ij")
    pnx = pnx.reshape(-1).astype(np.float32)
    pny = pny.reshape(-1).astype(np.float32)
    out = {}
    for a in ANGLES:
        ox, oy = _OFF[a // 45]
        dx = pnx + np.array(ox, dtype=np.float32)
        dy = pny + np.array(oy, dtype=np.float32)
        rows = []
        for n in range(9):
            Ax = int(np.floor(dx[n]))
            Ay = int(np.floor(dy[n]))
            fx = float(np.float32(dx[n] - Ax))
            fy = float(np.float32(dy[n] - Ay))
            rows.append((n, n // 3, n % 3, Ax, fx, Ay, fy))
        out[a] = rows
    return out


TABLES = _tables()
# distinct fractional row offsets shared by the 45/135 pair
LERP_DS = sorted({(t[3], t[4]) for a in (45, 135) for t in TABLES[a]})

# Integer-offset angles 90/180: per output phase (rho,sig), conv tap
# (ki,kj) reads slab offset (R,C). dx/dy for n=3r+s: 90 -> (s-1, 1-r),
# 180 -> (1-r, 1-s).
def _phase_taps(a):
    taps = {}
    for rho in range(3):
        for sig in range(3):
            lst = []
            for ki in range(3):
                ai, ri = divmod(rho + ki, 3)
                for kj in range(3):
                    bj, sj = divmod(sig + kj, 3)
                    if ri == 1 and sj == 1:
                        continue  # shared center tap
                    if a == 90:
                        R, C = ai + (sj - 1), bj + (1 - ri)
                    else:  # 180
                        R, C = ai + (1 - ri), bj + (1 - sj)
                    lst.append((3 * ki + kj, R, C))
            taps[(rho, sig)] = lst
    return taps


PHASE_TAPS = {90: _phase_taps(90), 180: _phase_taps(180)}

# The center sampling tap (ri=1, sj=1 -> n=4) reads x itself with the same
# weight kk and the same (ai,bj) window for EVERY angle: compute it once per
# (b, ot, phase) and fuse the add into each angle's PSUM evacuation.
def _center(rho, sig):
    ki, kj = (1 - rho) % 3, (1 - sig) % 3
    return 3 * ki + kj, (rho + ki) // 3, (sig + kj) // 3


CENTER = {(rho, sig): _center(rho, sig) for rho in range(3) for sig in range(3)}

# Angle-0 phase-collapsed conv: output phase rho uses row taps di with the
# listed combo of original kernel rows (g(m)=m//3+m%3-1 collides for m=1,3
# and m=2,4). Combo indices into the host-precomputed sums: 0,1,2 = single
# ki, 3 = ki0+ki2. Same structure for columns. 49 taps/phase-grid vs 81.
ROW_COMBOS = [(0,), (1,), (2,), (0, 2)]
PHROWS = {0: [(-1, 0), (0, 1), (1, 2)],
          1: [(0, 3), (1, 1)],
          2: [(1, 3), (0, 1)]}

N_WARM = 14        # warm-up matmuls: keep HAM at 8/8 through the DMA window
# valid output columns per column phase sig (cols sig::3 below 190)
NCOL = {0: 64, 1: 63, 2: 63}
SIG_OFF = {0: 0, 1: 8 * 64, 2: 8 * 64 + 8 * 63}  # sig block offsets in a rho block
ROWB = 8 * 190     # elements per rho block in packed staging (=sum of sig blocks)


def build_graph():
    nc = bacc.Bacc()
    # xc: per ct, the opening half-job's whole dependency set in ONE
    # DMA: [slab b0 | all 9 ot=0 weight slices]. The early window is
    # descriptor-rate limited; one descriptor per partition wins.
    xc = nc.declare_dram_parameter("xc", [2, 128, SLABF + 1152], BF16, False)
    xs1 = nc.declare_dram_parameter("xs1", [2, 128, SLABF], BF16, False)
    wt = nc.declare_dram_parameter("wt", [2, 128, 9 * 128], BF16, False)  # ot=1
    wc = nc.declare_dram_parameter("wc", [2, 128, 16 * 2 * 128], BF16, False)
    # flat per-(angle,b,ot) rows: packed rho-major [3,8,190] for the
    # slab-direct jobs (contiguous DMA), natural [24,190] for 45/135.
    out = nc.declare_dram_parameter("out", [5, 2, 2, 128, 3 * 8 * 190], BF16, True)

    with tile.TileContext(nc) as tc:
        with (
            tc.tile_pool(name="const", bufs=1) as constp,
            tc.tile_pool(name="cenp", bufs=1) as cenp,
            tc.tile_pool(name="xop", bufs=2) as xop,
            tc.tile_pool(name="rcp", bufs=1) as rcp,
            tc.tile_pool(name="stg", bufs=2) as stgp,
            tc.tile_pool(name="ps", bufs=8, space="PSUM") as psp,
        ):
            # HAM warm-up: dependency-free matmuls on a vector-memset tile
            # keep the PE busy during the input-DMA window so the clock
            # gate is already at 8/8 when the first real matmul issues.
            # N=256 keeps the handoff granularity small. Results land in
            # a scratch PSUM bank and are never read.
            warm = constp.tile([128, 384], BF16, name="warm", tag="warm")
            nc.gpsimd.memset(warm[:], 0.0)
            wps = psp.tile([128, 512], F32, name="wps", tag="ps")
            for _ in range(N_WARM):
                nc.tensor.matmul(wps[:, :256], warm[:, :128], warm[:, 128:384],
                                 start=True, stop=True)

            # DMA order matters for the head: the first job (slab-direct
            # angle-90, batch 0, ct-outer) can issue its first matmul once
            # slab b0/ct0 and the first wt chunk (ct0, ot0, kk 0-2) have
            # landed.
            slab = {}

            # Inputs split across BOTH hardware-DGE queues: sync carries
            # ct=0, the scalar engine's queue carries ct=1. The packed
            # critical xc goes first on each queue.
            eng = {0: nc.sync, 1: nc.scalar}
            xc_sb = []
            w_sb = []
            wc_sb = []
            for ct in range(2):
                t = constp.tile([128, SLABF + 1152], BF16,
                                name=f"xc{ct}", tag=f"xc{ct}")
                eng[ct].dma_start(t[:], xc[ct])
                xc_sb.append(t)
                slab[(0, ct)] = t[:, :SLABF]
                w_sb.append(constp.tile([128, 9 * 128], BF16,
                                        name=f"w{ct}", tag=f"w{ct}"))
            for ct in range(2):  # collapsed weights, ot=0 half first
                wctile = constp.tile([128, 16 * 2 * 128], BF16, name=f"wc{ct}",
                                     tag=f"wc{ct}")
                eng[ct].dma_start(wctile[:, :2048], wc[ct][:, :2048])
                wc_sb.append(wctile)
            for ct in range(2):  # ot=1 weights
                eng[ct].dma_start(w_sb[ct][:], wt[ct])
            for ct in range(2):  # batch-1 slabs
                s = constp.tile([128, SLABF], BF16,
                                name=f"slab1{ct}", tag=f"slab1{ct}")
                eng[ct].dma_start(s[:], xs1[ct])
                slab[(1, ct)] = s
            for ct in range(2):
                eng[ct].dma_start(wc_sb[ct][:, 2048:], wc[ct][:, 2048:])

            def slab3(b, ct):
                return slab[(b, ct)].rearrange("p (r c) -> p r c", c=SLAB_COLS)

            def wap(ct, ot, kk):
                if ot == 0:
                    return xc_sb[ct][:, SLABF + kk * 128 : SLABF + (kk + 1) * 128]
                return w_sb[ct][:, kk * 128 : (kk + 1) * 128]

            def build_lerp_rc(b):
                """Shared row-lerp R_d and col-diff C_d tiles for 45+135."""
                R = {}
                C = {}
                for ct in range(2):
                    sv = slab3(b, ct)
                    dr = rcp.tile([128, 12 * SLAB_COLS], BF16,
                                  name=f"dr{ct}", tag=f"dr{ct}")
                    drv = dr.rearrange("p (r c) -> p r c", c=SLAB_COLS)
                    nc.vector.tensor_tensor(drv, sv[:, 1:13, :], sv[:, 0:12, :],
                                            AluOpType.subtract)
                    for di, (Ax, fx) in enumerate(LERP_DS):
                        if fx == 0.0:
                            rv = sv[:, 2 + Ax : 11 + Ax, :]
                        else:
                            rt = rcp.tile([128, 9 * SLAB_COLS], BF16,
                                          name=f"r{ct}_{di}", tag=f"r{ct}_{di}")
                            rv = rt.rearrange("p (r c) -> p r c", c=SLAB_COLS)
                            nc.vector.scalar_tensor_tensor(
                                rv, drv[:, 2 + Ax : 11 + Ax, :], fx,
                                sv[:, 2 + Ax : 11 + Ax, :],
                                AluOpType.mult, AluOpType.add)
                        ctile = rcp.tile([128, 9 * SLAB_COLS], BF16,
                                         name=f"c{ct}_{di}", tag=f"c{ct}_{di}")
                        cv = ctile.rearrange("p (r c) -> p r c", c=SLAB_COLS)
                        nc.vector.tensor_tensor(cv[:, :, 0:69], rv[:, :, 1:70],
                                                rv[:, :, 0:69], AluOpType.subtract)
                        R[(ct, Ax, fx)] = rv
                        C[(ct, Ax, fx)] = cv
                return R, C

            def build_xo_lerp(a, b, R, C):
                """Phase-major sampled image: 9 contiguous blocks of
                [9 rows x 65 cols], one per sampling phase n, so the conv
                reads contiguous-inner APs and the builds write contiguous
                (fast DVE mode). Block r=2 only needs 8 rows."""
                xo = []
                for ct in range(2):
                    t = xop.tile([128, 9 * 585], BF16, name=f"xo{ct}", tag=f"xo{ct}")
                    for (n, r, s, Ax, fx, Ay, fy) in TABLES[a]:
                        if n == 4:
                            continue  # center block replaced by shared partial
                        nrow = 9 if r < 2 else 8
                        dst = t[:, n * 585 : n * 585 + nrow * 65].rearrange(
                            "p (r c) -> p r c", c=65)
                        rv = R[(ct, Ax, fx)][:, :nrow, 2 + Ay : 67 + Ay]
                        if fy == 0.0:
                            nc.vector.tensor_copy(dst, rv)
                        else:
                            cv = C[(ct, Ax, fx)][:, :nrow, 2 + Ay : 67 + Ay]
                            nc.vector.scalar_tensor_tensor(
                                dst, cv, fy, rv,
                                AluOpType.mult, AluOpType.add)
                    xo.append(t)
                return xo

            cen = {}

            def emit_center(b, ot):
                """Shared center-tap partials for one (b, ot): 9 phase
                windows, 2 ct matmuls each, Act-evacuated to SBUF f16."""
                t = cenp.tile([128, 9 * 512], BF16, name=f"cen{b}{ot}",
                              tag=f"cen{b}{ot}")
                for rho in range(3):
                    for sig in range(3):
                        kk, ai, bj = CENTER[(rho, sig)]
                        nn = NCOL[sig]
                        ps = psp.tile([128, 512], F32, name="cps", tag="ps")
                        for ct in range(2):
                            sv = slab3(b, ct)
                            nc.tensor.matmul(
                                ps[:, : 8 * nn], wap(ct, ot, kk),
                                sv[:, ai + 2 : ai + 10, bj + 2 : bj + 2 + nn],
                                start=(ct == 0), stop=(ct == 1))
                        o = (3 * rho + sig) * 512
                        nc.scalar.copy(t[:, o : o + 8 * nn], ps[:, : 8 * nn])
                cen[(b, ot)] = t

            def _evac_rho_cen(stg, ps, rho, dma_dst, cent):
                """Evacuate one rho group, fusing the shared center-tap add
                on the vector engine."""
                for sig in range(3):
                    nn = NCOL[sig]
                    o = rho * ROWB + SIG_OFF[sig]
                    co = (3 * rho + sig) * 512
                    nc.vector.tensor_tensor(
                        stg[:, o : o + 8 * nn], ps[sig][:, : 8 * nn],
                        cent[:, co : co + 8 * nn], AluOpType.add)
                nc.sync.dma_start(
                    dma_dst[:, rho * ROWB : (rho + 1) * ROWB],
                    stg[:, rho * ROWB : (rho + 1) * ROWB])

            def _evac_rho(stg, ps, rho, last, dma_dst):
                """Evacuate one rho group's 3 column phases into packed
                phase-major staging (contiguous copies — the host
                de-interleaves columns) and DMA the rho block. The closing
                job evacuates on the otherwise-idle vector engine and DMAs
                per sig block, so only the last sig's short chain trails
                the final matmul."""
                for sig in range(3):
                    nn = NCOL[sig]
                    o = rho * ROWB + SIG_OFF[sig]
                    dst = stg[:, o : o + 8 * nn]
                    if last:
                        nc.vector.tensor_copy(dst, ps[sig][:, : 8 * nn])
                        nc.sync.dma_start(dma_dst[:, o : o + 8 * nn], dst)
                    else:
                        nc.scalar.copy(dst, ps[sig][:, : 8 * nn])
                if not last:
                    nc.sync.dma_start(
                        dma_dst[:, rho * ROWB : (rho + 1) * ROWB],
                        stg[:, rho * ROWB : (rho + 1) * ROWB])

            def conv_job_angle0(ai, b, last=False):
                """Phase-collapsed conv reading the slab directly (no xo).
                Groups by output row phase rho so each rho's rows can DMA
                out as soon as its three column phases are evacuated."""
                for ot in range(2):
                    stg = stgp.tile([128, 3 * ROWB], BF16, name="stg", tag="stg")
                    for rho in range(3):
                        ps = {sig: psp.tile([128, 512], F32, name=f"ps{sig}",
                                            tag="ps") for sig in range(3)}
                        for sig in range(3):
                            nn = NCOL[sig]
                            taps = [(di, dj, ri * 4 + ci)
                                    for (di, ri) in PHROWS[rho]
                                    for (dj, ci) in PHROWS[sig]]
                            nmm = len(taps) * 2
                            i = 0
                            for ct in range(2):
                                sv = slab3(b, ct)
                                for (di, dj, cb) in taps:
                                    w_ap = wc_sb[ct][:, (ot * 16 + cb) * 128 :
                                                     (ot * 16 + cb + 1) * 128]
                                    nc.tensor.matmul(
                                        ps[sig][:, : 8 * nn], w_ap,
                                        sv[:, di + 2 : di + 10,
                                           dj + 2 : dj + 2 + nn],
                                        start=(i == 0), stop=(i == nmm - 1))
                                    i += 1
                        _evac_rho(stg, ps, rho, last, out[ai, b, ot])

            def conv_job_phase(ai, b, angle, emit_cen=False):
                """Slab-direct conv for integer-offset angles 90/180: each
                output phase is an 8-tap conv with remapped slab offsets
                plus the shared center partial added at evacuation."""
                ptaps = PHASE_TAPS[angle]
                for ot in range(2):
                    if emit_cen:
                        emit_center(b, ot)
                    stg = stgp.tile([128, 3 * ROWB], BF16, name="stg", tag="stg")
                    for rho in range(3):
                        ps = {sig: psp.tile([128, 512], F32, name=f"ps{sig}",
                                            tag="ps") for sig in range(3)}
                        for sig in range(3):
                            nn = NCOL[sig]
                            i = 0
                            for ct in range(2):
                                sv = slab3(b, ct)
                                for (kk, R, C) in ptaps[(rho, sig)]:
                                    w_ap = wap(ct, ot, kk)
                                    nc.tensor.matmul(
                                        ps[sig][:, : 8 * nn], w_ap,
                                        sv[:, R + 2 : R + 10, C + 2 : C + 2 + nn],
                                        start=(i == 0), stop=(i == 15))
                                    i += 1
                        _evac_rho_cen(stg, ps, rho, out[ai, b, ot],
                                      cen[(b, ot)])

            def conv_job(ai, b, xo):
                """Phase-structured conv on the phase-major sampled image;
                same shape as conv_job_phase but taps read xo blocks."""
                xov = [xo[ct].rearrange("p (n r c) -> p n r c", n=9, c=65)
                       for ct in range(2)]
                for ot in range(2):
                    stg = stgp.tile([128, 3 * ROWB], BF16, name="stg", tag="stg")
                    for rho in range(3):
                        ps = {sig: psp.tile([128, 512], F32, name=f"ps{sig}",
                                            tag="ps") for sig in range(3)}
                        for sig in range(3):
                            nn = NCOL[sig]
                            i = 0
                            for ct in range(2):
                                for ki in range(3):
                                    ai_, ri = divmod(rho + ki, 3)
                                    for kj in range(3):
                                        bj, sj = divmod(sig + kj, 3)
                                        if ri == 1 and sj == 1:
                                            continue  # shared center tap
                                        n = 3 * ri + sj
                                        kk = 3 * ki + kj
                                        w_ap = wap(ct, ot, kk)
                                        nc.tensor.matmul(
                                            ps[sig][:, : 8 * nn], w_ap,
                                            xov[ct][:, n, ai_ : ai_ + 8,
                                                    bj : bj + nn],
                                            start=(i == 0), stop=(i == 15))
                                        i += 1
                        _evac_rho_cen(stg, ps, rho, out[ai, b, ot],
                                      cen[(b, ot)])

            # Slab-direct angle 90 first (smallest DMA dependency set),
            # collapsed angle 0 last for b=1 so the tail is its staggered
            # per-rho DMAs.
            conv_job_phase(2, 0, 90, emit_cen=True)
            conv_job_angle0(0, 0)
            conv_job_phase(4, 0, 180)
            R, C = build_lerp_rc(0)
            xo = build_xo_lerp(45, 0, R, C)
            conv_job(1, 0, xo)
            xo = build_xo_lerp(135, 0, R, C)
            conv_job(3, 0, xo)
            conv_job_phase(2, 1, 90, emit_cen=True)
            conv_job_phase(4, 1, 180)
            R, C = build_lerp_rc(1)
            xo = build_xo_lerp(45, 1, R, C)
            conv_job(1, 1, xo)
            xo = build_xo_lerp(135, 1, R, C)
            conv_job(3, 1, xo)
            conv_job_angle0(0, 1, last=True)

    nc.compile()
    return nc


_GRAPH = None


def _graph():
    global _GRAPH
    if _GRAPH is None:
        _GRAPH = build_graph()
    return _GRAPH


def prep_inputs(x, weight):
    x = np.asarray(x, dtype=np.float32)
    weight = np.asarray(weight, dtype=np.float32)
    # pad data rows -2..66, cols -2..67
    xp = np.pad(x, ((0, 0), (0, 0), (2, 3), (2, 4))).astype(np.float16)
    w6 = weight.reshape(2, 128, 2, 128, 3, 3)                 # [ot,o,ct,c,ki,kj]
    # wt ot-major per ct: [ct, c, ot, ki, kj, o]
    w = w6.transpose(2, 3, 0, 4, 5, 1).reshape(2, 128, 2 * 9 * 128)
    w = np.ascontiguousarray(w.astype(np.float16))
    w0, w1 = w[:, :, :1152], np.ascontiguousarray(w[:, :, 1152:])
    xc_cores, xs1_cores = [], []
    for k in range(NCORES):
        sl = xp[:, :, 8 * k : 8 * k + SLAB_ROWS, :]          # [2,256,13,70]
        sl = sl.reshape(2, 2, 128, SLABF)
        xc_cores.append(np.ascontiguousarray(
            np.concatenate([sl[0], w0], axis=-1)))
        xs1_cores.append(np.ascontiguousarray(sl[1]))
    combos = []
    for Rc in ROW_COMBOS:
        for Cc in ROW_COMBOS:
            combos.append(w6[..., list(Rc), :][..., list(Cc)].sum(axis=(-1, -2)))
    wcarr = np.stack(combos, axis=0)                          # [16,ot,o,ct,c]
    wcarr = wcarr.transpose(3, 4, 1, 0, 2).reshape(2, 128, 2 * 16 * 128)
    wcarr = np.ascontiguousarray(wcarr.astype(np.float16))
    return xc_cores, xs1_cores, w1, wcarr


def assemble(results):
    full = np.empty((5, 2, 256, NCORES * NR, 190), np.float32)
    for k in range(NCORES):
        o = results[k]["out"]                       # [5,2,2,128,3*8*190] f16
        r0 = NR * k
        for ai in range(5):
            a = o[ai].reshape(2, 256, 3 * ROWB)
            # packed phase-major: [rho][sig-block][8 rows][ncol]
            for rho in range(3):
                for sig in range(3):
                    nn = NCOL[sig]
                    off = rho * ROWB + SIG_OFF[sig]
                    blk = a[:, :, off : off + 8 * nn].reshape(2, 256, 8, nn)
                    full[ai, :, :, r0 + rho : r0 + NR : 3, sig::3] = blk
    full = full[:, :, :, :190, :]
    return tuple(np.ascontiguousarray(full[i]) for i in range(5))


def run(x, weight, trace=False, **trace_kw):
    xc_cores, xs1_cores, w1, wcarr = prep_inputs(x, weight)
    nc = _graph()
    in_maps = [{"xc": xc_cores[k], "xs1": xs1_cores[k], "wt": w1, "wc": wcarr}
               for k in range(NCORES)]
    res = run_bass_kernel_spmd(nc, in_maps, core_ids=list(range(NCORES)),
                               trace=trace, **trace_kw)
    return assemble(res.results), res


def kernel(x, weight):
    return run(x, weight)[0]

